# revision 1
# baseline (speedup 1.0000x reference)
"""Trainium2 Bass kernel for a single transformer decoder layer.

Reference semantics (B=64, T=200, E=512, H=8, D=64):
  x += SelfAttn(LN1(x))   (q,k row-masked by pred_mask, causal)
  x += CrossAttn(LN2(x))  (k from raw memory row-masked by src_mask,
                           v from LN2(x) (!), causal)
  x += FFN(LN3(x))        (512 -> 2048 -> relu -> 512)

Sharding: data-parallel over batch, 8 elems per NeuronCore, no collectives.

Layout strategy (per core, batch elems processed in PAIRS):
  - residual stream x kept NATURAL [t_chunk<=128, 512] in fp32
  - LN via bn_stats/bn_aggr + two fused scalar_tensor_tensor ops
  - activations transposed to [E, 2*T] pair tiles via PE is_transpose
    matmuls (keeps PE warm), DVE drains the PSUM
  - Q,K projected transposed [H*D, 2*T] with weight stationaries, N=400
  - scores computed TRANSPOSED  ST[s, t] = K Q^T  per head per elem,
    2 heads per PSUM bank; exp on ACT (no max subtraction -- scores are
    O(1)); causal mask applied post-exp via gpsimd.affine_select(fill=0)
  - matmul operands must sit at SBUF base partition 0 (row-group-64
    operands crash the device), so odd heads read DMA-shifted copies
  - softmax denominators via one-hot-column matmuls into [8,T] PSUM;
    1/d via reciprocal_approx_fast, broadcast to head halves by a
    one-hot matmul, multiplied into O^T on DVE
  - AV gives O transposed directly (lhsT = V natural slices)
  - biases enter PSUM via rank-1 (K=1) matmuls; FFN b1 rides the
    relu activation bias (per-partition in the transposed layout)
"""

import numpy as np
import ml_dtypes
from contextlib import ExitStack

import concourse.bass as bass
import concourse.bacc as bacc
import concourse.tile as tile
from concourse import mybir
from concourse.bass_utils import run_bass_kernel_spmd

B, T, E, H, Dh, F = 64, 200, 512, 8, 64, 2048
NCORES = 8
SCALE = float(E) ** -0.5
F32 = mybir.dt.float32
BF16 = mybir.dt.bfloat16
AL = mybir.AluOpType
AF = mybir.ActivationFunctionType
TCH = [(0, 128), (128, 72)]  # token chunks (t0, tc)
ECH = E // 128  # 4
FCH = F // 128  # 16
NPBF16 = ml_dtypes.bfloat16

_programs = {}


def _layernorm(nc, pools, x_c, tc, eps):
    """x_c: [tc,512] f32 natural -> (x-mu)*rsqrt(var+eps) as bf16.
    LN gamma is folded into the downstream weights host-side; beta enters
    via rank-1 bias matmuls."""
    st6 = pools["small"].tile([tc, 6], F32, name="st6")
    nc.vector.bn_stats(st6[:, :], x_c)
    mv = pools["small"].tile([tc, 2], F32, name="mv")
    nc.vector.bn_aggr(mv[:, :], st6[:, :])
    std = pools["small"].tile([tc, 1], F32, name="std")
    nc.scalar.activation(std[:, :], mv[:, 1:2], AF.Sqrt, bias=eps[0:tc, 0:1])
    rstd = pools["small"].tile([tc, 1], F32, name="rstd")
    nc.vector.reciprocal(rstd[:, :], std[:, :])
    nb = pools["small"].tile([tc, 1], F32, name="nb")
    nc.vector.tensor_scalar(nb[:, :], mv[:, 0:1], rstd[:, 0:1], -1.0,
                            op0=AL.mult, op1=AL.mult)
    h_c = pools["h"].tile([tc, E], BF16, name="h_c", tag="h_c", bufs=6)
    nc.scalar.activation(h_c[:, :], x_c, AF.Identity, scale=rstd[:, 0:1],
                         bias=nb[:, 0:1])
    return h_c


def _transpose_pair(nc, pools, h_cs_pair, ident):
    """h_cs_pair: list of 2 elems x 2 chunks of [tc,512] bf16 natural ->
    hT[ec] [128, 400] bf16 pair tiles via PE transposes."""
    hT = []
    for ec in range(ECH):
        t = pools["tT"].tile([128, 2 * T], BF16, name="hT", bufs=6)
        for el in range(2):
            for ci, (t0, tc) in enumerate(TCH):
                ps = pools["ps"].tile([128, tc], BF16, name="t_ps", tag="ps")
                nc.tensor.transpose(
                    ps[:, :], h_cs_pair[el][ci][0:tc, ec * 128:(ec + 1) * 128],
                    ident[0:tc, 0:tc])
                nc.vector.tensor_copy(t[:, el * T + t0:el * T + t0 + tc], ps[:, :])
        hT.append(t)
    return hT


def _project_qkT(nc, pools, w_sb, rhs_T, name, brow=None, mrow=None):
    """[128, 400] bf16 pair chunks of (W^T h)^T, plus base-partition-0
    copies of rows 64:128 (odd heads must read from partition 0).
    brow: [1,512] LN-beta@W row, added as a rank-1 term (masked by mrow)."""
    out, hi = [], []
    for oc in range(4):
        ps = pools["ps"].tile([128, 2 * T], F32, name=f"{name}_ps", tag="ps")
        for ec in range(ECH):
            nc.tensor.matmul(ps[:, :], w_sb[:, ec, oc * 128:(oc + 1) * 128],
                             rhs_T[ec][:, :], start=(ec == 0),
                             stop=(ec == 3 and brow is None))
        if brow is not None:
            nc.tensor.matmul(ps[:, :], brow[0:1, oc * 128:(oc + 1) * 128],
                             mrow[0:1, :], start=False, stop=True)
        qk = "q" if name.startswith("q") else "k"
        sb = pools["qkt"].tile([128, 2 * T], BF16, name=f"{name}_sb", tag=qk, bufs=5)
        nc.vector.tensor_copy(sb[:, :], ps[:, :])
        hb = pools["qkt"].tile([64, 2 * T], BF16, name=f"{name}_hi", tag="hi",
                               bufs=10)
        nc.sync.dma_start(hb[:, :], sb[64:128, :])
        out.append(sb)
        hi.append(hb)
    return out, hi


def _project_v(nc, pools, wv_sb, hT, off, name, brow=None, ones_row=None):
    """v natural [tc, 512] bf16 tiles for ONE elem (lhsT = hT pair slices)."""
    out = []
    for (t0, tc) in TCH:
        ps = pools["ps"].tile([tc, E], F32, name=f"{name}_ps", tag="ps")
        for ec in range(ECH):
            nc.tensor.matmul(ps[:, :], hT[ec][:, off + t0:off + t0 + tc],
                             wv_sb[:, ec, :], start=(ec == 0),
                             stop=(ec == 3 and brow is None))
        if brow is not None:
            nc.tensor.matmul(ps[:, :], ones_row[0:1, 0:tc], brow[0:1, :],
                             start=False, stop=True)
        sb = pools["v"].tile([tc, E], BF16, name=f"{name}_sb", tag="v", bufs=6)
        nc.scalar.copy(sb[:, :], ps[:, :])
        out.append(sb)
    return out


def _attention(nc, pools, qkt, v_sb, sel_sb, selB, wo_sb, bo_row, ones_row,
               x_cs, off):
    """Causal attention for ONE elem (token cols off:off+200 of the pair
    tiles) + output projection + bias + residual."""
    (qT_lo, qT_hi), (kT_lo, kT_hi) = qkt
    e0m, e1m = [], []
    # pass A: scores (transposed), exp, causal select; 2 heads per psum bank
    for oc in range(4):
        st0 = pools["ps"].tile([128, 2, 200], F32, name="st0", tag="ps")
        st1 = pools["ps"].tile([72, 2, 72], F32, name="st1", tag="ps")
        for hl in range(2):
            qh = (qT_lo, qT_hi)[hl][oc][0:64, off:off + 200]
            kh = (kT_lo, kT_hi)[hl][oc][0:64, off:off + 200]
            nc.tensor.matmul(st0[:, hl, :], kh[:, 0:128], qh)
            nc.tensor.matmul(st1[:, hl, :], kh[:, 128:200], qh[:, 128:200])
        e0 = pools["e0"].tile([128, 2, 200], BF16, name="e0", bufs=3)
        nc.scalar.activation(e0[:, :, :], st0[:, :, :], AF.Exp, scale=SCALE)
        e1 = pools["e1"].tile([72, 2, 72], BF16, name="e1", bufs=3)
        nc.scalar.activation(e1[:, :, :], st1[:, :, :], AF.Exp, scale=SCALE)
        # causal: keep where t - s >= 0 (iota = -p + t), else 0
        e0x = pools["e0"].tile([128, 2, 200], BF16, name="e0x", bufs=5)
        nc.gpsimd.affine_select(
            e0x[:, :, :], e0[:, :, :], pattern=[[0, 2], [1, 200]],
            compare_op=AL.is_ge, fill=0.0, base=0, channel_multiplier=-1)
        e1x = pools["e1"].tile([72, 2, 72], BF16, name="e1x", bufs=5)
        nc.gpsimd.affine_select(
            e1x[:, :, :], e1[:, :, :], pattern=[[0, 2], [1, 72]],
            compare_op=AL.is_ge, fill=0.0, base=0, channel_multiplier=-1)
        e0m.append(e0x)
        e1m.append(e1x)
    # pass B: denominators d[h, t] = sum_s exp -- one-hot stationaries
    dT = pools["ps"].tile([8, 200], F32, name="dT", tag="ps")
    for oc in range(4):
        for hl in range(2):
            h = 2 * oc + hl
            nc.tensor.matmul(dT[:, 0:200], sel_sb[0:128, h, :], e0m[oc][:, hl, :],
                             start=(h == 0), stop=False, skip_group_check=True)
            nc.tensor.matmul(dT[:, 128:200], sel_sb[0:72, h, :], e1m[oc][:, hl, :],
                             start=False, stop=(h == 7), skip_group_check=True)
    dt_sb = pools["small"].tile([8, 200], F32, name="dt_sb")
    nc.vector.tensor_copy(dt_sb[:, :], dT[:, :])
    dinvT = pools["small"].tile([8, 200], F32, name="dinvT")
    nc.vector.reciprocal_approx_fast(dinvT[:, :], dt_sb[:, :])
    # pass C: O^T = V^T @ E^T, normalized by 1/d broadcast to head halves
    oT_sb = []
    for oc in range(4):
        dbc_ps = pools["ps"].tile([128, 200], F32, name="dbc_ps", tag="ps")
        nc.tensor.matmul(dbc_ps[:, :], selB[0:8, oc, :], dinvT[:, :])
        dbc = pools["dbc"].tile([128, 200], F32, name="dbc")
        nc.vector.tensor_copy(dbc[:, :], dbc_ps[:, :])
        ot_ps = pools["ps"].tile([128, 200], F32, name="ot_ps", tag="ps")
        for hl in range(2):
            h = 2 * oc + hl
            hp = hl * 64
            nc.tensor.matmul(ot_ps[hp:hp + 64, 0:200],
                             v_sb[0][0:128, h * 64:(h + 1) * 64],
                             e0m[oc][:, hl, :], start=True, stop=False,
                             skip_group_check=True)
            nc.tensor.matmul(ot_ps[hp:hp + 64, 128:200],
                             v_sb[1][0:72, h * 64:(h + 1) * 64],
                             e1m[oc][:, hl, :], start=False, stop=True,
                             skip_group_check=True)
        ot = pools["ot"].tile([128, 200], BF16, name="ot", bufs=6)
        nc.vector.tensor_mul(ot[:, :], ot_ps[:, :], dbc[:, :])
        oT_sb.append(ot)
    # output projection (natural) + bias via rank-1 matmul + residual
    new_x = []
    for ci, (t0, tc) in enumerate(TCH):
        ps = pools["ps"].tile([tc, E], F32, name="proj_ps", tag="ps")
        for hc in range(4):
            nc.tensor.matmul(ps[:, :], oT_sb[hc][:, t0:t0 + tc],
                             wo_sb[:, hc, :], start=(hc == 0), stop=False)
        nc.tensor.matmul(ps[:, :], ones_row[0:1, 0:tc], bo_row[0:1, :],
                         start=False, stop=True)
        xn = pools["res"].tile([tc, E], F32, name="xn", tag="res")
        nc.vector.tensor_add(xn[:, :], ps[:, :], x_cs[ci])
        new_x.append(xn)
    return new_x


def _build(bpc, stages=3):
    nc = bacc.Bacc("TRN2", target_bir_lowering=False, debug=False,
                   enable_asserts=False, num_devices=NCORES)
    dram = {}

    def din(name, shape, dt):
        h = nc.dram_tensor(name, list(shape), dt, kind="ExternalInput")
        dram[name] = h
        return h

    x_d = din("x", (bpc, T, E), F32)
    mem_d = din("mem", (bpc, T, E), BF16)
    pm_d = din("pm", (bpc, T), BF16)
    sm_d = din("sm", (bpc, T), BF16)
    wq_sa_d = din("wq_sa", (E, E), BF16)
    wk_sa_d = din("wk_sa", (E, E), BF16)
    wv_sa_d = din("wv_sa", (E, E), BF16)
    wo_sa_d = din("wo_sa", (E, E), BF16)
    bo_sa_d = din("bo_sa", (1, E), BF16)
    wq_ca_d = din("wq_ca", (E, E), BF16)
    wk_ca_d = din("wk_ca", (E, E), BF16)
    wv_ca_d = din("wv_ca", (E, E), BF16)
    wo_ca_d = din("wo_ca", (E, E), BF16)
    bo_ca_d = din("bo_ca", (1, E), BF16)
    w1_d = din("w1", (E, F), BF16)
    b1_d = din("b1", (1, F), BF16)
    w2_d = din("w2", (F, E), BF16)
    b2_d = din("b2", (1, E), BF16)
    bq_sa_d = din("bq_sa", (1, E), BF16)
    bk_sa_d = din("bk_sa", (1, E), BF16)
    bv_sa_d = din("bv_sa", (1, E), BF16)
    bq_ca_d = din("bq_ca", (1, E), BF16)
    bv_ca_d = din("bv_ca", (1, E), BF16)
    out_d = nc.dram_tensor("out", [bpc, T, E], F32, kind="ExternalOutput")

    sel_np = np.zeros((128, 8, 8), dtype=NPBF16)
    for h in range(8):
        sel_np[:, h, h] = 1
    sel_d = nc.inline_tensor(sel_np, name="selc")
    ones_d = nc.inline_tensor(np.ones((1, E), dtype=NPBF16), name="onesc")
    selB_np = np.zeros((8, 4, 128), dtype=np.float32)
    for oc in range(4):
        selB_np[2 * oc, oc, 0:64] = 1
        selB_np[2 * oc + 1, oc, 64:128] = 1
    selB_d = nc.inline_tensor(selB_np, name="selBc")
    identb_d = nc.inline_tensor(np.eye(128, dtype=NPBF16), name="identbc")

    with tile.TileContext(nc) as tcx, ExitStack() as ctx:
        pools = {}

        def pool(name, bufs, space="SBUF"):
            pools[name] = ctx.enter_context(
                tcx.tile_pool(name=name, bufs=bufs, space=space))
            return pools[name]

        wpool = pool("w", 1)
        pool("small", 6)
        pool("lnt", 3)
        pool("h", 6)
        pool("tT", 5)
        pool("qkt", 5)
        pool("v", 5)
        pool("e0", 3)
        pool("e1", 3)
        pool("ot", 6)
        pool("dbc", 3)
        pool("res", 12)
        pool("rT", 17)
        pool("mrow", 3)
        pool("mbc", 5)
        pool("ps", 8, space="PSUM")

        def wtile(name, src, shape, rearr=None, dt=BF16, eng=None):
            t = wpool.tile(shape, dt, tag=name, bufs=1, name=name)
            ap = src[:] if rearr is None else src[:].rearrange(rearr, p=128)
            (eng or nc.sync).dma_start(t[...], ap)
            return t

        # SA weights first (sync queue) so pair 0 starts quickly; bulk
        # FFN/CA weights go on the scalar HWDGE queue in parallel
        identb = wtile("identb", identb_d, [128, 128])
        sel_sb = wtile("sel", sel_d, [128, 8, 8])
        selB = wtile("selB", selB_d, [8, 4, 128], dt=F32)
        ones_row = wtile("ones", ones_d, [1, E])
        wq_sa = wtile("wq_sa", wq_sa_d, [128, ECH, E], "(c p) n -> p c n")
        wk_sa = wtile("wk_sa", wk_sa_d, [128, ECH, E], "(c p) n -> p c n")
        wv_sa = wtile("wv_sa", wv_sa_d, [128, ECH, E], "(c p) n -> p c n")
        wo_sa = wtile("wo_sa", wo_sa_d, [128, ECH, E], "(c p) n -> p c n")
        bq_sa = wtile("bq_sa", bq_sa_d, [1, E])
        bk_sa = wtile("bk_sa", bk_sa_d, [1, E])
        bv_sa = wtile("bv_sa", bv_sa_d, [1, E])
        bo_sa = wtile("bo_sa", bo_sa_d, [1, E])
        wq_ca = wtile("wq_ca", wq_ca_d, [128, ECH, E], "(c p) n -> p c n",
                      eng=nc.scalar)
        wk_ca = wtile("wk_ca", wk_ca_d, [128, ECH, E], "(c p) n -> p c n",
                      eng=nc.scalar)
        wv_ca = wtile("wv_ca", wv_ca_d, [128, ECH, E], "(c p) n -> p c n",
                      eng=nc.scalar)
        wo_ca = wtile("wo_ca", wo_ca_d, [128, ECH, E], "(c p) n -> p c n",
                      eng=nc.scalar)
        bq_ca = wtile("bq_ca", bq_ca_d, [1, E], eng=nc.scalar)
        bv_ca = wtile("bv_ca", bv_ca_d, [1, E], eng=nc.scalar)
        bo_ca = wtile("bo_ca", bo_ca_d, [1, E], eng=nc.scalar)
        w1 = wtile("w1", w1_d, [128, ECH, F], "(c p) n -> p c n", eng=nc.scalar)
        w2 = wtile("w2", w2_d, [128, FCH, E], "(c p) n -> p c n", eng=nc.scalar)
        b2r = wtile("b2", b2_d, [1, E], eng=nc.scalar)
        # f_b1 (+ folded ln3_b @ w1) in column layout for the relu bias
        b1c = wpool.tile([128, FCH], F32, tag="b1c", bufs=1, name="b1c")
        b1cb = wpool.tile([128, FCH], BF16, tag="b1cb", bufs=1, name="b1cb")
        nc.scalar.dma_start(b1cb[...],
                            b1_d[:].rearrange("o (c p) -> p (o c)", p=128))
        nc.vector.tensor_copy(b1c[:, :], b1cb[:, :])
        eps = wpool.tile([128, 1], F32, tag="eps", bufs=1, name="eps")
        nc.gpsimd.memset(eps[:, :], 1e-5)

        for pr in range(bpc // 2):
            els = (2 * pr, 2 * pr + 1)
            # ---- load x and masks for both elems ----
            x_el = []
            pm2 = pools["mbc"].tile([128, 2 * T], BF16, name="pm2")
            sm2 = pools["mbc"].tile([128, 2 * T], BF16, name="sm2")
            pmrow2 = pools["mrow"].tile([1, 2 * T], BF16, name="pmrow2", bufs=2)
            ones2 = pools["mrow"].tile([1, 2 * T], BF16, name="ones2", bufs=2)
            nc.gpsimd.memset(ones2[:, :], 1.0)
            for el, e in enumerate(els):
                x_cs = []
                for (t0, tc) in TCH:
                    xt = pools["res"].tile([tc, E], F32, name="x_in", tag="res")
                    nc.sync.dma_start(xt[:, :], x_d[e, t0:t0 + tc, :])
                    x_cs.append(xt)
                x_el.append(x_cs)
                nc.sync.dma_start(pmrow2[0:1, el * T:(el + 1) * T],
                                  pm_d[e:e + 1, :])
                nc.gpsimd.partition_broadcast(pm2[:, el * T:(el + 1) * T],
                                              pmrow2[0:1, el * T:(el + 1) * T])
                sm_row = pools["mrow"].tile([1, T], BF16, name="sm_row", bufs=2)
                nc.sync.dma_start(sm_row[:, :], sm_d[e:e + 1, :])
                nc.gpsimd.partition_broadcast(sm2[:, el * T:(el + 1) * T],
                                              sm_row[:, :])

            # ======== self-attention ========
            h_pair = [[_layernorm(nc, pools, x_el[el][ci][:, :], tc, eps)
                       for ci, (t0, tc) in enumerate(TCH)] for el in range(2)]
            hT = _transpose_pair(nc, pools, h_pair, identb)
            hmT = []
            for ec in range(ECH):
                m = pools["tT"].tile([128, 2 * T], BF16, name="hmT", bufs=5)
                nc.vector.tensor_mul(m[:, :], hT[ec][:, :], pm2[:, :])
                hmT.append(m)
            qT = _project_qkT(nc, pools, wq_sa, hmT, "q_sa", bq_sa, pmrow2)
            kT = _project_qkT(nc, pools, wk_sa, hmT, "k_sa", bk_sa, pmrow2)
            for el in range(2):
                v_sb = _project_v(nc, pools, wv_sa, hT, el * T, "v_sa",
                                  bv_sa, ones_row)
                x_el[el] = _attention(nc, pools, (qT, kT), v_sb, sel_sb, selB,
                                      wo_sa, bo_sa, ones_row, x_el[el], el * T)
            if stages == 1:
                for el, e in enumerate(els):
                    for ci, (t0, tc) in enumerate(TCH):
                        nc.sync.dma_start(out_d[e, t0:t0 + tc, :],
                                          x_el[el][ci][:, :])
                continue

            # ======== cross-attention ========
            h_pair = [[_layernorm(nc, pools, x_el[el][ci][:, :], tc, eps)
                       for ci, (t0, tc) in enumerate(TCH)] for el in range(2)]
            h2T = _transpose_pair(nc, pools, h_pair, identb)
            m_pair = []
            for el, e in enumerate(els):
                m_cs = []
                for (t0, tc) in TCH:
                    mt = pools["h"].tile([tc, E], BF16, name="m_nat",
                                         tag="m_nat", bufs=6)
                    nc.sync.dma_start(mt[:, :], mem_d[e, t0:t0 + tc, :])
                    m_cs.append(mt)
                m_pair.append(m_cs)
            mT = _transpose_pair(nc, pools, m_pair, identb)
            memT = []
            for ec in range(ECH):
                mm = pools["tT"].tile([128, 2 * T], BF16, name="memTm", bufs=5)
                nc.vector.tensor_mul(mm[:, :], mT[ec][:, :], sm2[:, :])
                memT.append(mm)
            qT = _project_qkT(nc, pools, wq_ca, h2T, "q_ca", bq_ca, ones2)
            kT = _project_qkT(nc, pools, wk_ca, memT, "k_ca")
            for el in range(2):
                v_sb = _project_v(nc, pools, wv_ca, h2T, el * T, "v_ca",
                                  bv_ca, ones_row)
                x_el[el] = _attention(nc, pools, (qT, kT), v_sb, sel_sb, selB,
                                      wo_ca, bo_ca, ones_row, x_el[el], el * T)
            if stages == 2:
                for el, e in enumerate(els):
                    for ci, (t0, tc) in enumerate(TCH):
                        nc.sync.dma_start(out_d[e, t0:t0 + tc, :],
                                          x_el[el][ci][:, :])
                continue

            # ======== feed-forward ========
            h_pair = [[_layernorm(nc, pools, x_el[el][ci][:, :], tc, eps)
                       for ci, (t0, tc) in enumerate(TCH)] for el in range(2)]
            h3T = _transpose_pair(nc, pools, h_pair, identb)
            rT = []
            for fc in range(FCH):
                zps = pools["ps"].tile([128, 2 * T], F32, name="z_ps",
                                          tag="ps")
                for ec in range(ECH):
                    nc.tensor.matmul(zps[:, :],
                                     w1[:, ec, fc * 128:(fc + 1) * 128],
                                     h3T[ec][:, :], start=(ec == 0),
                                     stop=(ec == 3))
                r = pools["rT"].tile([128, 2 * T], BF16, name="r")
                nc.scalar.activation(r[:, :], zps[:, :], AF.Relu,
                                     bias=b1c[:, fc:fc + 1])
                rT.append(r)
            for el, e in enumerate(els):
                for ci, (t0, tc) in enumerate(TCH):
                    yps = pools["ps"].tile([tc, E], F32, name="y_ps",
                                                tag="ps")
                    for fc in range(FCH):
                        nc.tensor.matmul(yps[:, :],
                                         rT[fc][:, el * T + t0:el * T + t0 + tc],
                                         w2[:, fc, :], start=(fc == 0),
                                         stop=False)
                    nc.tensor.matmul(yps[:, :], ones_row[0:1, 0:tc],
                                     b2r[0:1, :], start=False, stop=True)
                    yout = pools["res"].tile([tc, E], F32, name="yout",
                                             tag="res")
                    nc.vector.tensor_add(yout[:, :], yps[:, :],
                                         x_el[el][ci][:, :])
                    nc.sync.dma_start(out_d[e, t0:t0 + tc, :], yout[:, :])

    nc.compile()
    return nc


def _host_prep(inputs, bpc, core):
    """Build the in_map for one core."""
    s = slice(core * bpc, (core + 1) * bpc)

    def rearr(w, g=None):  # (H, E, D) -> [E, H*D], optionally row-scaled
        m = np.transpose(np.asarray(w, np.float32), (1, 0, 2)).reshape(E, E)
        if g is not None:
            m = m * np.asarray(g, np.float32)[:, None]
        return np.ascontiguousarray(m).astype(NPBF16)

    def b16(a):
        return np.ascontiguousarray(np.asarray(a, np.float32)).astype(NPBF16)

    def f32c(a):
        return np.ascontiguousarray(np.asarray(a, np.float32))

    g1 = np.asarray(inputs["ln1_g"], np.float32)
    b1n = np.asarray(inputs["ln1_b"], np.float32)
    g2 = np.asarray(inputs["ln2_g"], np.float32)
    b2n = np.asarray(inputs["ln2_b"], np.float32)
    g3 = np.asarray(inputs["ln3_g"], np.float32)
    b3n = np.asarray(inputs["ln3_b"], np.float32)

    def wr(w):  # raw rearranged fp32 (for beta @ W rows)
        return np.transpose(np.asarray(w, np.float32), (1, 0, 2)).reshape(E, E)

    return {
        "x": f32c(inputs["idx"][s]),
        "mem": b16(inputs["memory"][s]),
        "pm": b16(inputs["pred_mask"][s] != 0),
        "sm": b16(inputs["src_mask"][s] != 0),
        "wq_sa": rearr(inputs["sa_wq"], g1), "wk_sa": rearr(inputs["sa_wk"], g1),
        "wv_sa": rearr(inputs["sa_wv"], g1),
        "wo_sa": b16(inputs["sa_wo"]), "bo_sa": b16(inputs["sa_bo"]).reshape(1, E),
        "bq_sa": b16(b1n @ wr(inputs["sa_wq"])).reshape(1, E),
        "bk_sa": b16(b1n @ wr(inputs["sa_wk"])).reshape(1, E),
        "bv_sa": b16(b1n @ wr(inputs["sa_wv"])).reshape(1, E),
        "wq_ca": rearr(inputs["ca_wq"], g2), "wk_ca": rearr(inputs["ca_wk"]),
        "wv_ca": rearr(inputs["ca_wv"], g2),
        "wo_ca": b16(inputs["ca_wo"]), "bo_ca": b16(inputs["ca_bo"]).reshape(1, E),
        "bq_ca": b16(b2n @ wr(inputs["ca_wq"])).reshape(1, E),
        "bv_ca": b16(b2n @ wr(inputs["ca_wv"])).reshape(1, E),
        "w1": b16(np.asarray(inputs["f_w1"], np.float32)
                  * g3[:, None]),
        "b1": b16(np.asarray(inputs["f_b1"], np.float32)
                  + b3n @ np.asarray(inputs["f_w1"], np.float32)).reshape(1, F),
        "w2": b16(inputs["f_w2"]), "b2": b16(inputs["f_b2"]).reshape(1, E),
    }


def get_program(bpc):
    if bpc not in _programs:
        _programs[bpc] = _build(bpc)
    return _programs[bpc]


def kernel(**inputs) -> np.ndarray:
    bpc = B // NCORES
    nc = get_program(bpc)
    in_maps = [_host_prep(inputs, bpc, c) for c in range(NCORES)]
    res = run_bass_kernel_spmd(nc, in_maps, core_ids=list(range(NCORES)))
    out = np.concatenate([res.results[c]["out"] for c in range(NCORES)], axis=0)
    return out.astype(np.float32)



# revision 19
# speedup vs baseline: 1.1994x; 1.1994x over previous
"""Trainium2 Bass kernel for a single transformer decoder layer.

Reference semantics (B=64, T=200, E=512, H=8, D=64):
  x += SelfAttn(LN1(x))   (q,k row-masked by pred_mask, causal)
  x += CrossAttn(LN2(x))  (k from raw memory row-masked by src_mask,
                           v from LN2(x) (!), causal)
  x += FFN(LN3(x))        (512 -> 2048 -> relu -> 512)

Sharding: data-parallel over batch, 8 elems per NeuronCore, no collectives.

Layout strategy (per core, batch elems processed in PAIRS):
  - residual stream x kept NATURAL [t_chunk<=128, 512] in fp32
  - LN via bn_stats/bn_aggr; normalize-drain emits fp8e4 (x16 static
    scale) directly so all projection GEMMs run fp8 DoubleRow (2 k-tiles
    of 128 per matmul, 0.5 cyc/row = 2x bf16 PE throughput)
  - weights quantized host-side to fp8e4 with per-tensor power-of-2
    scales (baked into the build; program cache keyed on them)
  - activations transposed to [128, 2, 2*T] fp8 DR pair tiles via PE
    is_transpose matmuls, DVE drains PSUM into the interleaved views
  - Q,K drains fuse bias + pred-mask via scalar_tensor_tensor
    ((psum + bias) * mask); Q/K stay un-descaled in bf16 and the fp8
    scale factors fold into the softmax exp() scale
  - V bias (LN beta @ Wv) folds into the output-projection bias since
    softmax weights sum to one; V drains descale on ACT
  - scores computed TRANSPOSED per head in bf16, exp on ACT, causal
    mask via gpsimd.affine_select(fill=0) post-exp
  - softmax denominators via one-hot-column matmuls into [8,T] PSUM;
    1/d via reciprocal_approx_fast (bf16), broadcast to head halves by
    a bf16 one-hot matmul carrying the x16 fp8 ot scale
  - AV gives O transposed; the 1/d multiply emits fp8 into DR pair
    tiles for the output projection; residual-add drains descale via
    scalar_tensor_tensor ((psum * s) + x)
  - FFN relu rides the ACT drain (relu(s*psum + b)*16 via positive
    homogeneity) emitting fp8 DR pair tiles for W2
"""

import numpy as np
import ml_dtypes
from contextlib import ExitStack

import concourse.bass as bass
import concourse.bacc as bacc
import concourse.tile as tile
from concourse import mybir
from concourse.bass_utils import run_bass_kernel_spmd

B, T, E, H, Dh, F = 64, 200, 512, 8, 64, 2048
NCORES = 8
SCALE = float(E) ** -0.5
F32 = mybir.dt.float32
BF16 = mybir.dt.bfloat16
FP8 = mybir.dt.float8e4
AL = mybir.AluOpType
AF = mybir.ActivationFunctionType
DR = mybir.MatmulPerfMode.DoubleRow
TCH = [(0, 128), (128, 72)]  # token chunks (t0, tc)
EOFF = 208   # per-elem column offset in pair tiles (16B-aligned for fp8 LDW)
PT = 2 * EOFF  # pair-tile width
ECH = E // 128  # 4
FCH = F // 128  # 16
NPBF16 = ml_dtypes.bfloat16
NPFP8 = ml_dtypes.float8_e4m3fn
SH = 16.0   # static fp8 scale for LN outputs / memory
SOT = 16.0  # static fp8 scale for attention output (rides selB)
SRT = 16.0  # static fp8 scale for relu output

_programs = {}


def _layernorm(nc, pools, x_c, tc, eps):
    """x_c: [tc,512] f32 natural -> (x-mu)*rsqrt(var+eps)*SH as fp8e4.
    LN gamma is folded into the downstream weights host-side; beta enters
    via per-partition bias columns on the Q/K drains (V beta folds into
    the output-projection bias)."""
    st6 = pools["small"].tile([tc, 6], F32, name="st6")
    nc.vector.bn_stats(st6[:, :], x_c)
    mv = pools["small"].tile([tc, 2], F32, name="mv")
    nc.vector.bn_aggr(mv[:, :], st6[:, :])
    # std/SH = sqrt((var + eps)/SH^2)
    std = pools["small"].tile([tc, 1], F32, name="std")
    nc.scalar.activation(std[:, :], mv[:, 1:2], AF.Sqrt, bias=eps[0:tc, 0:1],
                         scale=1.0 / (SH * SH))
    rstd = pools["small"].tile([tc, 1], F32, name="rstd")  # = SH/std
    nc.vector.reciprocal(rstd[:, :], std[:, :])
    nb = pools["small"].tile([tc, 1], F32, name="nb")
    nc.vector.tensor_scalar(nb[:, :], mv[:, 0:1], rstd[:, 0:1], -1.0,
                            op0=AL.mult, op1=AL.mult)
    h_c = pools["h"].tile([tc, E], BF16, name="h_c", tag="h_c", bufs=6)
    nc.scalar.activation(h_c[:, :], x_c, AF.Identity, scale=rstd[:, 0:1],
                         bias=nb[:, 0:1])
    return h_c


def _transpose_pair(nc, pools, h_cs_pair, ident, dt):
    """h_cs_pair: 2 elems x 2 chunks of [tc,512] bf16 natural -> 2 DR
    pair tiles [128, 2, 2*T] of dtype dt (k-tiles (2j, 2j+1)
    interleaved) via bf16 PE transposes; the drain casts."""
    hT = []
    for j in range(2):
        t = pools["tT"].tile([128, 2, PT], dt, name="hT", bufs=6)
        for i in range(2):
            ec = 2 * j + i
            for el in range(2):
                for ci, (t0, tc) in enumerate(TCH):
                    ps = pools["ps"].tile([128, tc], BF16, name="t_ps", tag="ps")
                    nc.tensor.transpose(
                        ps[:, :], h_cs_pair[el][ci][0:tc, ec * 128:(ec + 1) * 128],
                        ident[0:tc, 0:tc])
                    nc.vector.tensor_copy(
                        t[:, i, el * EOFF + t0:el * EOFF + t0 + tc], ps[:, :])
        hT.append(t)
    return hT


def _project_qkT(nc, pools, w_sb, rhs_T, name, bcol=None, mask=None):
    """fp8 DR projection -> [128, 400] bf16 pair chunks of (W^T h)^T,
    drain fuses per-partition bias add and token mask multiply.  Also
    makes base-partition-0 copies of rows 64:128 (odd heads must read
    matmul operands from partition 0)."""
    out, hi = [], []
    for oc in range(4):
        ps = pools["ps"].tile([128, PT], F32, name=f"{name}_ps", tag="ps")
        for j in range(2):
            nc.tensor.matmul(ps[:, :], w_sb[:, j, :, oc * 128:(oc + 1) * 128],
                             rhs_T[j][:, :, :], start=(j == 0), stop=(j == 1),
                             perf_mode=DR)
        qk = "q" if name.startswith("q") else "k"
        sb = pools["qkt"].tile([128, PT], BF16, name=f"{name}_sb", tag=qk, bufs=5)
        if mask is not None:
            nc.vector.scalar_tensor_tensor(
                sb[:, :], ps[:, :], bcol[:, oc:oc + 1] if bcol is not None else 0.0,
                mask[:, :], op0=AL.add, op1=AL.mult)
        elif bcol is not None:
            nc.vector.tensor_scalar(sb[:, :], ps[:, :], bcol[:, oc:oc + 1],
                                    None, op0=AL.add)
        else:
            nc.vector.tensor_copy(sb[:, :], ps[:, :])
        hb = pools["qkt"].tile([64, PT], BF16, name=f"{name}_hi", tag="hi",
                               bufs=10)
        nc.sync.dma_start(hb[:, :], sb[64:128, :])
        out.append(sb)
        hi.append(hb)
    return out, hi


def _project_v(nc, pools, wv_sb, hT, off, name, dsv):
    """v natural [tc, 512] bf16 tiles for ONE elem via fp8 DR
    (stationary = hT pair slices); ACT drain descales."""
    out = []
    for (t0, tc) in TCH:
        ps = pools["ps"].tile([tc, E], F32, name=f"{name}_ps", tag="ps")
        for j in range(2):
            nc.tensor.matmul(ps[:, :], hT[j][:, :, off + t0:off + t0 + tc],
                             wv_sb[:, j, :, :], start=(j == 0), stop=(j == 1),
                             perf_mode=DR)
        sb = pools["v"].tile([tc, E], BF16, name=f"{name}_sb", tag="v", bufs=6)
        nc.scalar.mul(sb[:, :], ps[:, :], dsv)
        out.append(sb)
    return out


def _attention(nc, pools, qkt, v_sb, sel_sb, selB, wo_sb, bo_row, ones_row,
               x_cs, off, exp_scale, dso):
    """Causal attention for ONE elem (token cols off:off+200 of the pair
    tiles) + fp8 DR output projection + bias + residual."""
    (qT_lo, qT_hi), (kT_lo, kT_hi) = qkt
    e0m, e1m = [], []
    # pass A: scores (transposed), exp, causal select; 2 heads per psum bank
    for oc in range(4):
        st0 = pools["ps"].tile([128, 2, 200], F32, name="st0", tag="ps")
        st1 = pools["ps"].tile([72, 2, 72], F32, name="st1", tag="ps")
        for hl in range(2):
            qh = (qT_lo, qT_hi)[hl][oc][0:64, off:off + 200]
            kh = (kT_lo, kT_hi)[hl][oc][0:64, off:off + 200]
            nc.tensor.matmul(st0[:, hl, :], kh[:, 0:128], qh)
            nc.tensor.matmul(st1[:, hl, :], kh[:, 128:200], qh[:, 128:200])
        e0 = pools["e0"].tile([128, 2, 200], BF16, name="e0", bufs=3)
        nc.scalar.activation(e0[:, :, :], st0[:, :, :], AF.Exp, scale=exp_scale)
        e1 = pools["e1"].tile([72, 2, 72], BF16, name="e1", bufs=3)
        nc.scalar.activation(e1[:, :, :], st1[:, :, :], AF.Exp, scale=exp_scale)
        # causal: keep where t - s >= 0 (iota = -p + t), else 0
        e0x = pools["e0"].tile([128, 2, 200], BF16, name="e0x", bufs=5)
        nc.gpsimd.affine_select(
            e0x[:, :, :], e0[:, :, :], pattern=[[0, 2], [1, 200]],
            compare_op=AL.is_ge, fill=0.0, base=0, channel_multiplier=-1)
        e1x = pools["e1"].tile([72, 2, 72], BF16, name="e1x", bufs=5)
        nc.gpsimd.affine_select(
            e1x[:, :, :], e1[:, :, :], pattern=[[0, 2], [1, 72]],
            compare_op=AL.is_ge, fill=0.0, base=0, channel_multiplier=-1)
        e0m.append(e0x)
        e1m.append(e1x)
    # pass B: denominators d[h, t] = sum_s exp -- one-hot stationaries
    dT = pools["ps"].tile([8, 200], F32, name="dT", tag="ps")
    for oc in range(4):
        for hl in range(2):
            h = 2 * oc + hl
            nc.tensor.matmul(dT[:, 0:200], sel_sb[0:128, h, :], e0m[oc][:, hl, :],
                             start=(h == 0), stop=False, skip_group_check=True)
            nc.tensor.matmul(dT[:, 128:200], sel_sb[0:72, h, :], e1m[oc][:, hl, :],
                             start=False, stop=(h == 7), skip_group_check=True)
    dinvf = pools["small"].tile([8, 200], F32, name="dinvf")
    nc.vector.reciprocal_approx_fast(dinvf[:, :], dT[:, :])
    dinvT = pools["small"].tile([8, 200], BF16, name="dinvT")
    nc.vector.tensor_copy(dinvT[:, :], dinvf[:, :])
    # pass C: O^T = V^T @ E^T, normalized by SOT/d broadcast to head halves
    oT_sb = [pools["ot"].tile([128, 2, EOFF], FP8, name="ot", bufs=4)
             for _ in range(2)]
    for oc in range(4):
        dbc_ps = pools["ps"].tile([128, 200], F32, name="dbc_ps", tag="ps")
        nc.tensor.matmul(dbc_ps[:, :], selB[0:8, oc, :], dinvT[:, :])
        dbc = pools["dbc"].tile([128, 200], BF16, name="dbc")
        nc.vector.tensor_copy(dbc[:, :], dbc_ps[:, :])
        ot_ps = pools["ps"].tile([128, 200], F32, name="ot_ps", tag="ps")
        for hl in range(2):
            h = 2 * oc + hl
            hp = hl * 64
            nc.tensor.matmul(ot_ps[hp:hp + 64, 0:200],
                             v_sb[0][0:128, h * 64:(h + 1) * 64],
                             e0m[oc][:, hl, :], start=True, stop=False,
                             skip_group_check=True)
            nc.tensor.matmul(ot_ps[hp:hp + 64, 128:200],
                             v_sb[1][0:72, h * 64:(h + 1) * 64],
                             e1m[oc][:, hl, :], start=False, stop=True,
                             skip_group_check=True)
        nc.vector.tensor_mul(oT_sb[oc // 2][:, oc % 2, 0:200], ot_ps[:, :],
                             dbc[:, :])
    # fp8 DR output projection + bias via rank-1 matmul + descaled residual
    new_x = []
    for ci, (t0, tc) in enumerate(TCH):
        ps = pools["ps"].tile([tc, E], F32, name="proj_ps", tag="ps")
        for j in range(2):
            nc.tensor.matmul(ps[:, :], oT_sb[j][:, :, t0:t0 + tc],
                             wo_sb[:, j, :, :], start=(j == 0), stop=False,
                             perf_mode=DR)
        nc.tensor.matmul(ps[:, :], ones_row[0:1, 0:tc], bo_row[0:1, :],
                         start=False, stop=True)
        xn = pools["res"].tile([tc, E], F32, name="xn", tag="res")
        nc.vector.scalar_tensor_tensor(xn[:, :], ps[:, :], dso, x_cs[ci],
                                       op0=AL.mult, op1=AL.add)
        new_x.append(xn)
    return new_x


def _build(bpc, scales, stages=3):
    (swq_sa, swk_sa, swv_sa, swo_sa, swq_ca, swk_ca, swv_ca, swo_ca,
     sw1, sw2) = scales
    nc = bacc.Bacc("TRN2", target_bir_lowering=False, debug=False,
                   enable_asserts=False, num_devices=NCORES)
    dram = {}

    def din(name, shape, dt):
        h = nc.dram_tensor(name, list(shape), dt, kind="ExternalInput")
        dram[name] = h
        return h

    x_d = din("x", (bpc, T, E), F32)
    mem_d = din("mem", (bpc, T, E), BF16)
    pm_d = din("pm", (bpc, T), BF16)
    sm_d = din("sm", (bpc, T), BF16)
    # fp8 DR weights, host layout [128, j, i, N]: row k = (2j+i)*128 + p
    wq_sa_d = din("wq_sa", (128, 2, 2, E), FP8)
    wk_sa_d = din("wk_sa", (128, 2, 2, E), FP8)
    wv_sa_d = din("wv_sa", (128, 2, 2, E), FP8)
    wo_sa_d = din("wo_sa", (128, 2, 2, E), FP8)
    bo_sa_d = din("bo_sa", (1, E), BF16)
    wq_ca_d = din("wq_ca", (128, 2, 2, E), FP8)
    wk_ca_d = din("wk_ca", (128, 2, 2, E), FP8)
    wv_ca_d = din("wv_ca", (128, 2, 2, E), FP8)
    wo_ca_d = din("wo_ca", (128, 2, 2, E), FP8)
    bo_ca_d = din("bo_ca", (1, E), BF16)
    w1_d = din("w1", (128, 2, 2, F), FP8)
    b1_d = din("b1", (1, F), F32)
    w2_d = din("w2", (128, 8, 2, E), FP8)
    b2_d = din("b2", (1, E), BF16)
    # per-partition bias columns (beta @ W, pre-scaled), [128, oc]
    bq_sa_d = din("bq_sa", (128, 4), F32)
    bk_sa_d = din("bk_sa", (128, 4), F32)
    bq_ca_d = din("bq_ca", (128, 4), F32)
    out_d = nc.dram_tensor("out", [bpc, T, E], F32, kind="ExternalOutput")

    sel_np = np.zeros((128, 8, 8), dtype=NPBF16)
    for h in range(8):
        sel_np[:, h, h] = 1
    sel_d = nc.inline_tensor(sel_np, name="selc")
    ones_d = nc.inline_tensor(np.ones((1, E), dtype=NPBF16), name="onesc")
    selB_np = np.zeros((8, 4, 128), dtype=NPBF16)
    for oc in range(4):
        selB_np[2 * oc, oc, 0:64] = SOT
        selB_np[2 * oc + 1, oc, 64:128] = SOT
    selB_d = nc.inline_tensor(selB_np, name="selBc")
    identb_d = nc.inline_tensor(np.eye(128, dtype=NPBF16), name="identbc")

    # drain descale immediates
    dsv_sa = 1.0 / (SH * swv_sa)
    dsv_ca = 1.0 / (SH * swv_ca)
    dso_sa = 1.0 / (SOT * swo_sa)
    dso_ca = 1.0 / (SOT * swo_ca)
    exp_sa = SCALE / (SH * SH * swq_sa * swk_sa)
    exp_ca = SCALE / (SH * SH * swq_ca * swk_ca)
    relu_s = SRT / (SH * sw1)
    ds2 = 1.0 / (SRT * sw2)

    with tile.TileContext(nc) as tcx, ExitStack() as ctx:
        pools = {}

        def pool(name, bufs, space="SBUF"):
            pools[name] = ctx.enter_context(
                tcx.tile_pool(name=name, bufs=bufs, space=space))
            return pools[name]

        wpool = pool("w", 1)
        pool("small", 6)
        pool("lnt", 3)
        pool("h", 6)
        pool("tT", 5)
        pool("qkt", 5)
        pool("v", 5)
        pool("e0", 3)
        pool("e1", 3)
        pool("ot", 4)
        pool("dbc", 3)
        pool("res", 12)
        pool("rT", 9)
        pool("mrow", 3)
        pool("mbc", 5)
        pool("ps", 8, space="PSUM")

        def wtile(name, src, shape, dt=FP8, eng=None):
            t = wpool.tile(shape, dt, tag=name, bufs=1, name=name)
            (eng or nc.sync).dma_start(t[...], src[:])
            return t

        # SA weights first (sync queue) so pair 0 starts quickly; bulk
        # FFN/CA weights go on the scalar HWDGE queue in parallel
        identb = wtile("identb", identb_d, [128, 128], dt=BF16)
        sel_sb = wtile("sel", sel_d, [128, 8, 8], dt=BF16)
        selB = wtile("selB", selB_d, [8, 4, 128], dt=BF16)
        ones_row = wtile("ones", ones_d, [1, E], dt=BF16)
        wq_sa = wtile("wq_sa", wq_sa_d, [128, 2, 2, E])
        wk_sa = wtile("wk_sa", wk_sa_d, [128, 2, 2, E])
        wv_sa = wtile("wv_sa", wv_sa_d, [128, 2, 2, E])
        wo_sa = wtile("wo_sa", wo_sa_d, [128, 2, 2, E])
        bq_sa = wtile("bq_sa", bq_sa_d, [128, 4], dt=F32)
        bk_sa = wtile("bk_sa", bk_sa_d, [128, 4], dt=F32)
        bo_sa = wtile("bo_sa", bo_sa_d, [1, E], dt=BF16)
        wq_ca = wtile("wq_ca", wq_ca_d, [128, 2, 2, E], eng=nc.scalar)
        wk_ca = wtile("wk_ca", wk_ca_d, [128, 2, 2, E], eng=nc.scalar)
        wv_ca = wtile("wv_ca", wv_ca_d, [128, 2, 2, E], eng=nc.scalar)
        wo_ca = wtile("wo_ca", wo_ca_d, [128, 2, 2, E], eng=nc.scalar)
        bq_ca = wtile("bq_ca", bq_ca_d, [128, 4], dt=F32, eng=nc.scalar)
        bo_ca = wtile("bo_ca", bo_ca_d, [1, E], dt=BF16, eng=nc.scalar)
        w1 = wtile("w1", w1_d, [128, 2, 2, F], eng=nc.scalar)
        w2 = wtile("w2", w2_d, [128, 8, 2, E], eng=nc.scalar)
        b2r = wtile("b2", b2_d, [1, E], dt=BF16, eng=nc.scalar)
        # f_b1 (+ folded ln3_b @ w1, pre-scaled by SRT) in column layout
        b1c = wpool.tile([128, FCH], F32, tag="b1c", bufs=1, name="b1c")
        nc.scalar.dma_start(b1c[...],
                            b1_d[:].rearrange("o (c p) -> p (o c)", p=128))
        eps = wpool.tile([128, 1], F32, tag="eps", bufs=1, name="eps")
        nc.gpsimd.memset(eps[:, :], 1e-5 / (SH * SH))

        for pr in range(bpc // 2):
            els = (2 * pr, 2 * pr + 1)
            # ---- load x and masks for both elems ----
            x_el = []
            pm2 = pools["mbc"].tile([128, PT], BF16, name="pm2")
            sm2 = pools["mbc"].tile([128, PT], BF16, name="sm2")
            pmrow2 = pools["mrow"].tile([1, PT], BF16, name="pmrow2", bufs=2)
            for el, e in enumerate(els):
                x_cs = []
                for (t0, tc) in TCH:
                    xt = pools["res"].tile([tc, E], F32, name="x_in", tag="res")
                    nc.sync.dma_start(xt[:, :], x_d[e, t0:t0 + tc, :])
                    x_cs.append(xt)
                x_el.append(x_cs)
                nc.sync.dma_start(pmrow2[0:1, el * EOFF:el * EOFF + T],
                                  pm_d[e:e + 1, :])
                nc.gpsimd.partition_broadcast(pm2[:, el * EOFF:el * EOFF + T],
                                              pmrow2[0:1, el * EOFF:el * EOFF + T])
                sm_row = pools["mrow"].tile([1, T], BF16, name="sm_row", bufs=2)
                nc.sync.dma_start(sm_row[:, :], sm_d[e:e + 1, :])
                nc.gpsimd.partition_broadcast(sm2[:, el * EOFF:el * EOFF + T],
                                              sm_row[:, :])

            # ======== self-attention ========
            h_pair = [[_layernorm(nc, pools, x_el[el][ci][:, :], tc, eps)
                       for ci, (t0, tc) in enumerate(TCH)] for el in range(2)]
            hT = _transpose_pair(nc, pools, h_pair, identb, FP8)
            qT = _project_qkT(nc, pools, wq_sa, hT, "q_sa", bq_sa, pm2)
            kT = _project_qkT(nc, pools, wk_sa, hT, "k_sa", bk_sa, pm2)
            for el in range(2):
                v_sb = _project_v(nc, pools, wv_sa, hT, el * EOFF, "v_sa",
                                  dsv_sa)
                x_el[el] = _attention(nc, pools, (qT, kT), v_sb, sel_sb, selB,
                                      wo_sa, bo_sa, ones_row, x_el[el],
                                      el * EOFF, exp_sa, dso_sa)
            if stages == 1:
                for el, e in enumerate(els):
                    for ci, (t0, tc) in enumerate(TCH):
                        nc.sync.dma_start(out_d[e, t0:t0 + tc, :],
                                          x_el[el][ci][:, :])
                continue

            # ======== cross-attention ========
            h_pair = [[_layernorm(nc, pools, x_el[el][ci][:, :], tc, eps)
                       for ci, (t0, tc) in enumerate(TCH)] for el in range(2)]
            h2T = _transpose_pair(nc, pools, h_pair, identb, FP8)
            m_pair = []
            for el, e in enumerate(els):
                m_cs = []
                for (t0, tc) in TCH:
                    mt = pools["h"].tile([tc, E], BF16, name="m_nat",
                                         tag="m_nat", bufs=6)
                    nc.sync.dma_start(mt[:, :], mem_d[e, t0:t0 + tc, :])
                    m_cs.append(mt)
                m_pair.append(m_cs)
            mT = _transpose_pair(nc, pools, m_pair, identb, BF16)
            # masked fp8 cast: memT = (mT * SH) * sm  into DR pair views
            memT = []
            for j in range(2):
                mm = pools["tT"].tile([128, 2, PT], FP8, name="memTm", bufs=5)
                for i in range(2):
                    nc.vector.scalar_tensor_tensor(
                        mm[:, i, :], mT[j][:, i, :], SH, sm2[:, :],
                        op0=AL.mult, op1=AL.mult)
                memT.append(mm)
            qT = _project_qkT(nc, pools, wq_ca, h2T, "q_ca", bq_ca, None)
            kT = _project_qkT(nc, pools, wk_ca, memT, "k_ca", None, None)
            for el in range(2):
                v_sb = _project_v(nc, pools, wv_ca, h2T, el * EOFF, "v_ca",
                                  dsv_ca)
                x_el[el] = _attention(nc, pools, (qT, kT), v_sb, sel_sb, selB,
                                      wo_ca, bo_ca, ones_row, x_el[el],
                                      el * EOFF, exp_ca, dso_ca)
            if stages == 2:
                for el, e in enumerate(els):
                    for ci, (t0, tc) in enumerate(TCH):
                        nc.sync.dma_start(out_d[e, t0:t0 + tc, :],
                                          x_el[el][ci][:, :])
                continue

            # ======== feed-forward ========
            h_pair = [[_layernorm(nc, pools, x_el[el][ci][:, :], tc, eps)
                       for ci, (t0, tc) in enumerate(TCH)] for el in range(2)]
            h3T = _transpose_pair(nc, pools, h_pair, identb, FP8)
            rT = [pools["rT"].tile([128, 2, PT], FP8, name="r")
                  for _ in range(FCH // 2)]
            for fc in range(FCH):
                zps = pools["ps"].tile([128, PT], F32, name="z_ps",
                                       tag="ps")
                for j in range(2):
                    nc.tensor.matmul(zps[:, :],
                                     w1[:, j, :, fc * 128:(fc + 1) * 128],
                                     h3T[j][:, :, :], start=(j == 0),
                                     stop=(j == 1), perf_mode=DR)
                # relu(z)*SRT = relu(SRT*z); bias pre-scaled by SRT host-side
                nc.scalar.activation(rT[fc // 2][:, fc % 2, :], zps[:, :],
                                     AF.Relu, bias=b1c[:, fc:fc + 1],
                                     scale=relu_s)
            for el, e in enumerate(els):
                for ci, (t0, tc) in enumerate(TCH):
                    yps = pools["ps"].tile([tc, E], F32, name="y_ps",
                                           tag="ps")
                    for j in range(FCH // 2):
                        nc.tensor.matmul(
                            yps[:, :],
                            rT[j][:, :, el * EOFF + t0:el * EOFF + t0 + tc],
                            w2[:, j, :, :], start=(j == 0), stop=False,
                            perf_mode=DR)
                    nc.tensor.matmul(yps[:, :], ones_row[0:1, 0:tc],
                                     b2r[0:1, :], start=False, stop=True)
                    yout = pools["res"].tile([tc, E], F32, name="yout",
                                             tag="res")
                    nc.vector.scalar_tensor_tensor(
                        yout[:, :], yps[:, :], ds2, x_el[el][ci][:, :],
                        op0=AL.mult, op1=AL.add)
                    nc.sync.dma_start(out_d[e, t0:t0 + tc, :], yout[:, :])

    nc.compile()
    return nc


def _pow2_scale(w):
    a = float(np.max(np.abs(np.asarray(w, np.float32))))
    if a == 0.0 or not np.isfinite(a):
        return 1.0
    return float(2.0 ** np.floor(np.log2(224.0 / a)))


def _weight_scales(inputs):
    g1 = np.asarray(inputs["ln1_g"], np.float32)[:, None]
    g2 = np.asarray(inputs["ln2_g"], np.float32)[:, None]
    g3 = np.asarray(inputs["ln3_g"], np.float32)[:, None]

    def wr(w):  # (H, E, D) -> [E, H*D]
        return np.transpose(np.asarray(w, np.float32), (1, 0, 2)).reshape(E, E)

    raw = {
        "wq_sa": wr(inputs["sa_wq"]), "wk_sa": wr(inputs["sa_wk"]),
        "wv_sa": wr(inputs["sa_wv"]),
        "wq_ca": wr(inputs["ca_wq"]), "wv_ca": wr(inputs["ca_wv"]),
        "w1": np.asarray(inputs["f_w1"], np.float32),
    }
    mats = {
        "wq_sa": raw["wq_sa"] * g1, "wk_sa": raw["wk_sa"] * g1,
        "wv_sa": raw["wv_sa"] * g1,
        "wo_sa": np.asarray(inputs["sa_wo"], np.float32),
        "wq_ca": raw["wq_ca"] * g2, "wk_ca": wr(inputs["ca_wk"]),
        "wv_ca": raw["wv_ca"] * g2,
        "wo_ca": np.asarray(inputs["ca_wo"], np.float32),
        "w1": raw["w1"] * g3,
        "w2": np.asarray(inputs["f_w2"], np.float32),
    }
    order = ["wq_sa", "wk_sa", "wv_sa", "wo_sa", "wq_ca", "wk_ca", "wv_ca",
             "wo_ca", "w1", "w2"]
    scales = tuple(_pow2_scale(mats[k]) for k in order)
    return mats, raw, dict(zip(order, scales)), scales


def _host_prep(inputs, mats, raw, sc, bpc, core):
    """Build the in_map for one core."""
    s = slice(core * bpc, (core + 1) * bpc)

    def dr4(m, scale):  # [E, N] -> [128, 2, 2, N] fp8 DR layout
        q = (m * scale).reshape(2, 2, 128, -1).transpose(2, 0, 1, 3)
        return np.ascontiguousarray(q).astype(NPFP8)

    def dr_w2(m, scale):  # [F, E] -> [128, 8, 2, E] fp8 DR layout
        q = (m * scale).reshape(8, 2, 128, E).transpose(2, 0, 1, 3)
        return np.ascontiguousarray(q).astype(NPFP8)

    def bcol(v, scale):  # [E] row -> [128, 4] col layout, scaled
        return np.ascontiguousarray(
            (v * scale).reshape(4, 128).T).astype(np.float32)

    def b16(a):
        return np.ascontiguousarray(np.asarray(a, np.float32)).astype(NPBF16)

    b1n = np.asarray(inputs["ln1_b"], np.float32)
    b2n = np.asarray(inputs["ln2_b"], np.float32)
    b3n = np.asarray(inputs["ln3_b"], np.float32)

    # V beta-bias folds into the output-projection bias (softmax rows sum
    # to 1, so a constant V shift passes through attention unchanged)
    bo_sa_eff = (np.asarray(inputs["sa_bo"], np.float32)
                 + (b1n @ raw["wv_sa"]) @ mats["wo_sa"])
    bo_ca_eff = (np.asarray(inputs["ca_bo"], np.float32)
                 + (b2n @ raw["wv_ca"]) @ mats["wo_ca"])

    return {
        "x": np.ascontiguousarray(np.asarray(inputs["idx"], np.float32)[s]),
        "mem": b16(inputs["memory"][s]),
        "pm": b16(np.asarray(inputs["pred_mask"])[s] != 0),
        "sm": b16(np.asarray(inputs["src_mask"])[s] != 0),
        "wq_sa": dr4(mats["wq_sa"], sc["wq_sa"]),
        "wk_sa": dr4(mats["wk_sa"], sc["wk_sa"]),
        "wv_sa": dr4(mats["wv_sa"], sc["wv_sa"]),
        "wo_sa": dr4(mats["wo_sa"], sc["wo_sa"]),
        "bo_sa": np.ascontiguousarray(
            bo_sa_eff * (SOT * sc["wo_sa"])).reshape(1, E).astype(NPBF16),
        "bq_sa": bcol(b1n @ raw["wq_sa"], SH * sc["wq_sa"]),
        "bk_sa": bcol(b1n @ raw["wk_sa"], SH * sc["wk_sa"]),
        "wq_ca": dr4(mats["wq_ca"], sc["wq_ca"]),
        "wk_ca": dr4(mats["wk_ca"], sc["wk_ca"]),
        "wv_ca": dr4(mats["wv_ca"], sc["wv_ca"]),
        "wo_ca": dr4(mats["wo_ca"], sc["wo_ca"]),
        "bo_ca": np.ascontiguousarray(
            bo_ca_eff * (SOT * sc["wo_ca"])).reshape(1, E).astype(NPBF16),
        "bq_ca": bcol(b2n @ raw["wq_ca"], SH * sc["wq_ca"]),
        "w1": dr4(mats["w1"], sc["w1"]),
        "b1": np.ascontiguousarray(
            (np.asarray(inputs["f_b1"], np.float32) + b3n @ raw["w1"])
            * SRT).reshape(1, F).astype(np.float32),
        "w2": dr_w2(mats["w2"], sc["w2"]),
        "b2": np.ascontiguousarray(
            np.asarray(inputs["f_b2"], np.float32)
            * (SRT * sc["w2"])).reshape(1, E).astype(NPBF16),
    }


def get_program(bpc, scales):
    key = (bpc, scales)
    if key not in _programs:
        _programs[key] = _build(bpc, scales)
    return _programs[key]


def kernel(**inputs) -> np.ndarray:
    bpc = B // NCORES
    mats, raw, sc, scales = _weight_scales(inputs)
    nc = get_program(bpc, scales)
    in_maps = [_host_prep(inputs, mats, raw, sc, bpc, c)
               for c in range(NCORES)]
    res = run_bass_kernel_spmd(nc, in_maps, core_ids=list(range(NCORES)))
    out = np.concatenate([res.results[c]["out"] for c in range(NCORES)], axis=0)
    return out.astype(np.float32)


# revision 27
# speedup vs baseline: 1.4383x; 1.1992x over previous
"""Trainium2 Bass kernel for a single transformer decoder layer.

Reference semantics (B=64, T=200, E=512, H=8, D=64):
  x += SelfAttn(LN1(x))   (q,k row-masked by pred_mask, causal)
  x += CrossAttn(LN2(x))  (k from raw memory row-masked by src_mask,
                           v from LN2(x) (!), causal)
  x += FFN(LN3(x))        (512 -> 2048 -> relu -> 512)

Sharding: data-parallel over batch, 8 elems per NeuronCore, no collectives.

Layout strategy (per core, batch elems processed in PAIRS):
  - residual stream x kept NATURAL [t_chunk<=128, 512] in fp32
  - LN via bn_stats/bn_aggr; normalize-drain emits fp8e4 (x16 static
    scale) directly so all projection GEMMs run fp8 DoubleRow (2 k-tiles
    of 128 per matmul, 0.5 cyc/row = 2x bf16 PE throughput)
  - weights quantized host-side to fp8e4 with per-tensor power-of-2
    scales (baked into the build; program cache keyed on them)
  - activations transposed to [128, 2, 2*T] fp8 DR pair tiles via PE
    is_transpose matmuls, DVE drains PSUM into the interleaved views
  - Q,K drains fuse bias + pred-mask via scalar_tensor_tensor
    ((psum + bias) * mask); Q/K stay un-descaled in bf16 and the fp8
    scale factors fold into the softmax exp() scale
  - V bias (LN beta @ Wv) folds into the output-projection bias since
    softmax weights sum to one; V drains descale on ACT
  - scores computed TRANSPOSED per head in bf16, exp on ACT, causal
    mask via gpsimd.affine_select(fill=0) post-exp
  - softmax denominators via one-hot-column matmuls into [8,T] PSUM;
    1/d via reciprocal_approx_fast (bf16), broadcast to head halves by
    a bf16 one-hot matmul carrying the x16 fp8 ot scale
  - AV gives O transposed; the 1/d multiply emits fp8 into DR pair
    tiles for the output projection; residual-add drains descale via
    scalar_tensor_tensor ((psum * s) + x)
  - FFN relu rides the ACT drain (relu(s*psum + b)*16 via positive
    homogeneity) emitting fp8 DR pair tiles for W2
"""

import numpy as np
import ml_dtypes
from contextlib import ExitStack

import concourse.bass as bass
import concourse.bacc as bacc
import concourse.tile as tile
from concourse import mybir
from concourse.bass_utils import run_bass_kernel_spmd

B, T, E, H, Dh, F = 64, 200, 512, 8, 64, 2048
NCORES = 8
SCALE = float(E) ** -0.5
F32 = mybir.dt.float32
BF16 = mybir.dt.bfloat16
FP8 = mybir.dt.float8e4
AL = mybir.AluOpType
AF = mybir.ActivationFunctionType
DR = mybir.MatmulPerfMode.DoubleRow
TCH = [(0, 128), (128, 72)]  # token chunks (t0, tc)
EOFF = 208   # per-elem column offset in pair tiles (16B-aligned for fp8 LDW)
PT = 2 * EOFF  # pair-tile width
ECH = E // 128  # 4
FCH = F // 128  # 16
NPBF16 = ml_dtypes.bfloat16
NPFP8 = ml_dtypes.float8_e4m3fn
SH = 16.0   # static fp8 scale for LN outputs / memory
SOT = 16.0  # static fp8 scale for attention output (rides selB)
SRT = 16.0  # static fp8 scale for relu output

_programs = {}


def _layernorm(nc, pools, x_c, tc, eps):
    """x_c: [tc,512] f32 natural -> (x-mu)*rsqrt(var+eps)*SH as fp8e4.
    LN gamma is folded into the downstream weights host-side; beta enters
    via per-partition bias columns on the Q/K drains (V beta folds into
    the output-projection bias)."""
    st6 = pools["small"].tile([tc, 6], F32, name="st6")
    nc.vector.bn_stats(st6[:, :], x_c)
    mv = pools["small"].tile([tc, 2], F32, name="mv")
    nc.vector.bn_aggr(mv[:, :], st6[:, :])
    # std/SH = sqrt((var + eps)/SH^2)
    std = pools["small"].tile([tc, 1], F32, name="std")
    nc.scalar.activation(std[:, :], mv[:, 1:2], AF.Sqrt, bias=eps[0:tc, 0:1],
                         scale=1.0 / (SH * SH))
    rstd = pools["small"].tile([tc, 1], F32, name="rstd")  # = SH/std
    nc.vector.reciprocal(rstd[:, :], std[:, :])
    nb = pools["small"].tile([tc, 1], F32, name="nb")
    nc.vector.tensor_scalar(nb[:, :], mv[:, 0:1], rstd[:, 0:1], -1.0,
                            op0=AL.mult, op1=AL.mult)
    h_c = pools["h"].tile([tc, E], BF16, name="h_c", tag="h_c", bufs=6)
    nc.gpsimd.tensor_scalar(h_c[:, :], x_c, rstd[:, 0:1], nb[:, 0:1],
                            op0=AL.mult, op1=AL.add)
    return h_c


def _transpose_pair(nc, pools, h_cs_pair, ident, dt):
    """h_cs_pair: 2 elems x 2 chunks of [tc,512] bf16 natural -> 2 DR
    pair tiles [128, 2, 2*T] of dtype dt (k-tiles (2j, 2j+1)
    interleaved) via bf16 PE transposes; the drain casts."""
    hT = []
    for j in range(2):
        t = pools["tT"].tile([128, 2, PT], dt, name="hT", bufs=6)
        for i in range(2):
            ec = 2 * j + i
            ps = pools["ps"].tile([128, PT], BF16, name="t_ps", tag="ps")
            for el in range(2):
                for ci, (t0, tc) in enumerate(TCH):
                    o = el * EOFF + t0
                    nc.tensor.transpose(
                        ps[:, o:o + tc],
                        h_cs_pair[el][ci][0:tc, ec * 128:(ec + 1) * 128],
                        ident[0:tc, 0:tc])
            nc.vector.tensor_copy(t[:, i, :], ps[:, :])
        hT.append(t)
    return hT


def _project_qkT(nc, pools, w_sb, rhs_T, name, bcol=None, mask=None):
    """fp8 DR projection -> [128, 400] bf16 pair chunks of (W^T h)^T,
    drain fuses per-partition bias add and token mask multiply.  Also
    makes base-partition-0 copies of rows 64:128 (odd heads must read
    matmul operands from partition 0)."""
    out, hi = [], []
    for oc in range(4):
        ps = pools["ps"].tile([128, PT], F32, name=f"{name}_ps", tag="ps")
        for j in range(2):
            nc.tensor.matmul(ps[:, :], w_sb[:, j, :, oc * 128:(oc + 1) * 128],
                             rhs_T[j][:, :, :], start=(j == 0), stop=(j == 1),
                             perf_mode=DR)
        qk = "q" if name.startswith("q") else "k"
        sb = pools["qkt"].tile([128, PT], BF16, name=f"{name}_sb", tag=qk, bufs=5)
        if mask is not None:
            nc.vector.scalar_tensor_tensor(
                sb[:, :], ps[:, :], bcol[:, oc:oc + 1] if bcol is not None else 0.0,
                mask[:, :], op0=AL.add, op1=AL.mult)
        elif bcol is not None:
            nc.vector.tensor_scalar(sb[:, :], ps[:, :], bcol[:, oc:oc + 1],
                                    None, op0=AL.add)
        else:
            nc.vector.tensor_copy(sb[:, :], ps[:, :])
        hb = pools["qkt"].tile([64, PT], BF16, name=f"{name}_hi", tag="hi",
                               bufs=10)
        nc.sync.dma_start(hb[:, :], sb[64:128, :])
        out.append(sb)
        hi.append(hb)
    return out, hi


def _project_v(nc, pools, wv_sb, hT, off, name, dsv):
    """v for ONE elem as an fp8 DR pair tile [128, 2(s-chunk), 512] via
    fp8 DR projection (stationary = hT pair slices); ACT drains descale
    and rescale by SH for the fp8 AV matmuls; pad s-rows zeroed."""
    vt = pools["v"].tile([128, 2, E], FP8, name=f"{name}_sb", tag="v", bufs=6)
    nc.gpsimd.memset(vt[:, 1, :], 0.0)
    for si, (t0, tc) in enumerate(TCH):
        ps = pools["ps"].tile([tc, E], F32, name=f"{name}_ps", tag="ps")
        for j in range(2):
            nc.tensor.matmul(ps[:, :], hT[j][:, :, off + t0:off + t0 + tc],
                             wv_sb[:, j, :, :], start=(j == 0), stop=(j == 1),
                             perf_mode=DR)
        nc.scalar.mul(vt[0:tc, si, :], ps[:, :], dsv)
    return vt


def _attention(nc, pools, qkt, v_els, sel8, selB, wo_sb, bo_row, ones_row,
               x_els, exp_scale, dso, has_bo, ln64):
    """Causal attention for BOTH elems of a pair, phases interleaved so
    PE keeps busy while ACT/gpsimd run exp/masking.  e is fp8 (x64 via
    the exp bias); denominators and AV run fp8 DoubleRow over the two
    s-chunks.  fp8 DR output projection + optional bias + descaled
    residual add."""
    (qT_lo, qT_hi), (kT_lo, kT_hi) = qkt
    em_els = []
    # pass A: scores (transposed), exp -> fp8, causal select; per elem
    for el in range(2):
        off = el * EOFF
        em_oc = []
        for oc in range(4):
            st0 = pools["ps"].tile([128, 2, 200], F32, name="st0", tag="ps")
            st1 = pools["ps"].tile([72, 2, 72], F32, name="st1", tag="ps")
            for hl in range(2):
                qh = (qT_lo, qT_hi)[hl][oc][0:64, off:off + 200]
                kh = (kT_lo, kT_hi)[hl][oc][0:64, off:off + 200]
                nc.tensor.matmul(st0[:, hl, :], kh[:, 0:128], qh)
                nc.tensor.matmul(st1[:, hl, :], kh[:, 128:200], qh[:, 128:200])
            er = pools["e0"].tile([128, 2, 2, EOFF], FP8, name="er", bufs=4)
            em = pools["e1"].tile([128, 2, 2, EOFF], FP8, name="em", bufs=10)
            nc.scalar.activation(er[:, :, 0, 0:200], st0[:, :, :], AF.Exp,
                                 scale=exp_scale, bias=ln64[0:128, 0:1])
            nc.scalar.activation(er[0:72, :, 1, 128:200], st1[:, :, :], AF.Exp,
                                 scale=exp_scale, bias=ln64[0:72, 0:1])
            # zero the si=1 plane first: exp/select only write its valid
            # [0:72, 128:200] block, and DR AV/denoms need clean zeros in
            # the pad rows/cols (fp8 garbage could be NaN)
            nc.gpsimd.memset(em[:, :, 1, :], 0.0)
            nc.gpsimd.affine_select(
                em[:, :, 0, 0:200], er[:, :, 0, 0:200],
                pattern=[[0, 2], [1, 200]],
                compare_op=AL.is_ge, fill=0.0, base=0, channel_multiplier=-1)
            nc.gpsimd.affine_select(
                em[0:72, :, 1, 128:200], er[0:72, :, 1, 128:200],
                pattern=[[0, 2], [1, 72]],
                compare_op=AL.is_ge, fill=0.0, base=0, channel_multiplier=-1)
            em_oc.append(em)
        em_els.append(em_oc)
    # pass B: denominators d8[h, t] = sum_s e8 -- one-hot DR stationaries
    dinv_els = []
    for el in range(2):
        dT = pools["ps"].tile([8, 200], F32, name="dT", tag="ps")
        for oc in range(4):
            for hl in range(2):
                h = 2 * oc + hl
                nc.tensor.matmul(dT[:, :], sel8[:, h, :, 0:8],
                                 em_els[el][oc][:, hl, :, 0:200],
                                 start=(h == 0), stop=(h == 7),
                                 perf_mode=DR, skip_group_check=True)
        dinvf = pools["small"].tile([8, 200], F32, name="dinvf")
        nc.vector.reciprocal_approx_fast(dinvf[:, :], dT[:, :])
        dinvT = pools["small"].tile([8, 200], BF16, name="dinvT")
        nc.vector.tensor_copy(dinvT[:, :], dinvf[:, :])
        dinv_els.append(dinvT)
    # pass C: O^T = V^T E^T via DR over s-chunks; normalize by SOT/(64 d)
    oT_els = []
    for el in range(2):
        oT_sb = [pools["ot"].tile([128, 2, EOFF], FP8, name="ot", bufs=4)
                 for _ in range(2)]
        for oc in range(4):
            dbc_ps = pools["ps"].tile([128, 200], F32, name="dbc_ps", tag="ps")
            nc.tensor.matmul(dbc_ps[:, :], selB[0:8, oc, :], dinv_els[el][:, :])
            dbc = pools["dbc"].tile([128, 200], BF16, name="dbc")
            nc.vector.tensor_copy(dbc[:, :], dbc_ps[:, :])
            ot_ps = pools["ps"].tile([128, 200], F32, name="ot_ps", tag="ps")
            em = em_els[el][oc]
            vt = v_els[el]
            h = 2 * oc
            # even head: fp8 DR over both s-chunks (out at partition 0)
            nc.tensor.matmul(ot_ps[0:64, :], vt[:, :, h * 64:(h + 1) * 64],
                             em[:, 0, :, 0:200],
                             perf_mode=DR, skip_group_check=True)
            # odd head: out at partition 64 -- DR disallowed there, use
            # two plain fp8 matmuls (one per s-chunk)
            nc.tensor.matmul(ot_ps[64:128, 0:200],
                             vt[:, 0, (h + 1) * 64:(h + 2) * 64],
                             em[:, 1, 0, 0:200], start=True, stop=False,
                             skip_group_check=True)
            nc.tensor.matmul(ot_ps[64:128, 128:200],
                             vt[0:72, 1, (h + 1) * 64:(h + 2) * 64],
                             em[0:72, 1, 1, 128:200], start=False, stop=True,
                             skip_group_check=True)
            nc.vector.tensor_mul(oT_sb[oc // 2][:, oc % 2, 0:200], ot_ps[:, :],
                                 dbc[:, :])
        oT_els.append(oT_sb)
    # fp8 DR output projection (+ bias rank-1 if nonzero) + residual
    new_x = []
    for el in range(2):
        x_cs = x_els[el]
        nx = []
        for ci, (t0, tc) in enumerate(TCH):
            ps = pools["ps"].tile([tc, E], F32, name="proj_ps", tag="ps")
            for j in range(2):
                nc.tensor.matmul(ps[:, :], oT_els[el][j][:, :, t0:t0 + tc],
                                 wo_sb[:, j, :, :], start=(j == 0),
                                 stop=(j == 1 and not has_bo), perf_mode=DR)
            if has_bo:
                nc.tensor.matmul(ps[:, :], ones_row[0:1, 0:tc], bo_row[0:1, :],
                                 start=False, stop=True)
            xn = pools["res"].tile([tc, E], F32, name="xn", tag="res")
            nc.vector.scalar_tensor_tensor(xn[:, :], ps[:, :], dso, x_cs[ci],
                                           op0=AL.mult, op1=AL.add)
            nx.append(xn)
        new_x.append(nx)
    return new_x


def _build(bpc, scales, flags, stages=3):
    (swq_sa, swk_sa, swv_sa, swo_sa, swq_ca, swk_ca, swv_ca, swo_ca,
     sw1, sw2) = scales
    has_bq_sa, has_bk_sa, has_bo_sa, has_bq_ca, has_bo_ca, has_b2 = flags
    nc = bacc.Bacc("TRN2", target_bir_lowering=False, debug=False,
                   enable_asserts=False, num_devices=NCORES)
    dram = {}

    def din(name, shape, dt):
        h = nc.dram_tensor(name, list(shape), dt, kind="ExternalInput")
        dram[name] = h
        return h

    x_d = din("x", (bpc, T, E), F32)
    mem_d = din("mem", (bpc, T, E), BF16)
    pm_d = din("pm", (bpc, T), BF16)
    sm_d = din("sm", (bpc, T), BF16)
    # fp8 DR weights, host layout [128, j, i, N]: row k = (2j+i)*128 + p
    wq_sa_d = din("wq_sa", (128, 2, 2, E), FP8)
    wk_sa_d = din("wk_sa", (128, 2, 2, E), FP8)
    wv_sa_d = din("wv_sa", (128, 2, 2, E), FP8)
    wo_sa_d = din("wo_sa", (128, 2, 2, E), FP8)
    bo_sa_d = din("bo_sa", (1, E), BF16)
    wq_ca_d = din("wq_ca", (128, 2, 2, E), FP8)
    wk_ca_d = din("wk_ca", (128, 2, 2, E), FP8)
    wv_ca_d = din("wv_ca", (128, 2, 2, E), FP8)
    wo_ca_d = din("wo_ca", (128, 2, 2, E), FP8)
    bo_ca_d = din("bo_ca", (1, E), BF16)
    w1_d = din("w1", (128, 2, 2, F), FP8)
    b1_d = din("b1", (1, F), F32)
    w2_d = din("w2", (128, 8, 2, E), FP8)
    b2_d = din("b2", (1, E), BF16)
    # per-partition bias columns (beta @ W, pre-scaled), [128, oc]
    bq_sa_d = din("bq_sa", (128, 4), F32)
    bk_sa_d = din("bk_sa", (128, 4), F32)
    bq_ca_d = din("bq_ca", (128, 4), F32)
    out_d = nc.dram_tensor("out", [bpc, T, E], F32, kind="ExternalOutput")

    sel_np = np.zeros((128, 8, 2, 16), dtype=NPFP8)
    for h in range(8):
        sel_np[:, h, :, h] = 1
    sel_d = nc.inline_tensor(sel_np, name="selc")
    ones_d = nc.inline_tensor(np.ones((1, E), dtype=NPBF16), name="onesc")
    # dbc = (SOT/SH) * 1/(64 d): ot_ps carries SH*64, ot wants SOT
    selB_np = np.zeros((8, 4, 128), dtype=NPBF16)
    for oc in range(4):
        selB_np[2 * oc, oc, 0:64] = SOT / SH
        selB_np[2 * oc + 1, oc, 64:128] = SOT / SH
    selB_d = nc.inline_tensor(selB_np, name="selBc")
    identb_d = nc.inline_tensor(np.eye(128, dtype=NPBF16), name="identbc")

    # drain descale immediates (v keeps a factor SH for its fp8 cast)
    dsv_sa = 1.0 / swv_sa
    dsv_ca = 1.0 / swv_ca
    dso_sa = 1.0 / (SOT * swo_sa)
    dso_ca = 1.0 / (SOT * swo_ca)
    exp_sa = SCALE / (SH * SH * swq_sa * swk_sa)
    exp_ca = SCALE / (SH * SH * swq_ca * swk_ca)
    relu_s = SRT / (SH * sw1)
    ds2 = 1.0 / (SRT * sw2)

    with tile.TileContext(nc) as tcx, ExitStack() as ctx:
        pools = {}

        def pool(name, bufs, space="SBUF"):
            pools[name] = ctx.enter_context(
                tcx.tile_pool(name=name, bufs=bufs, space=space))
            return pools[name]

        wpool = pool("w", 1)
        pool("small", 6)
        pool("lnt", 3)
        pool("h", 6)
        pool("tT", 5)
        pool("qkt", 5)
        pool("v", 5)
        pool("e0", 3)
        pool("e1", 3)
        pool("ot", 4)
        pool("dbc", 3)
        pool("res", 12)
        pool("rT", 9)
        pool("mrow", 3)
        pool("mbc", 5)
        pool("ps", 8, space="PSUM")

        def wtile(name, src, shape, dt=FP8, eng=None):
            t = wpool.tile(shape, dt, tag=name, bufs=1, name=name)
            (eng or nc.sync).dma_start(t[...], src[:])
            return t

        # SA weights first (sync queue) so pair 0 starts quickly; bulk
        # FFN/CA weights go on the scalar HWDGE queue in parallel
        identb = wtile("identb", identb_d, [128, 128], dt=BF16)
        sel_sb = wtile("sel", sel_d, [128, 8, 2, 16])
        selB = wtile("selB", selB_d, [8, 4, 128], dt=BF16)
        ones_row = wtile("ones", ones_d, [1, E], dt=BF16)
        wq_sa = wtile("wq_sa", wq_sa_d, [128, 2, 2, E])
        wk_sa = wtile("wk_sa", wk_sa_d, [128, 2, 2, E])
        wv_sa = wtile("wv_sa", wv_sa_d, [128, 2, 2, E])
        wo_sa = wtile("wo_sa", wo_sa_d, [128, 2, 2, E])
        bq_sa = wtile("bq_sa", bq_sa_d, [128, 4], dt=F32)
        bk_sa = wtile("bk_sa", bk_sa_d, [128, 4], dt=F32)
        bo_sa = wtile("bo_sa", bo_sa_d, [1, E], dt=BF16)
        wq_ca = wtile("wq_ca", wq_ca_d, [128, 2, 2, E], eng=nc.scalar)
        wk_ca = wtile("wk_ca", wk_ca_d, [128, 2, 2, E], eng=nc.scalar)
        wv_ca = wtile("wv_ca", wv_ca_d, [128, 2, 2, E], eng=nc.scalar)
        wo_ca = wtile("wo_ca", wo_ca_d, [128, 2, 2, E], eng=nc.scalar)
        bq_ca = wtile("bq_ca", bq_ca_d, [128, 4], dt=F32, eng=nc.scalar)
        bo_ca = wtile("bo_ca", bo_ca_d, [1, E], dt=BF16, eng=nc.scalar)
        w1 = wtile("w1", w1_d, [128, 2, 2, F], eng=nc.scalar)
        w2 = wtile("w2", w2_d, [128, 8, 2, E], eng=nc.scalar)
        b2r = wtile("b2", b2_d, [1, E], dt=BF16, eng=nc.scalar)
        # f_b1 (+ folded ln3_b @ w1, pre-scaled by SRT) in column layout
        b1c = wpool.tile([128, FCH], F32, tag="b1c", bufs=1, name="b1c")
        nc.scalar.dma_start(b1c[...],
                            b1_d[:].rearrange("o (c p) -> p (o c)", p=128))
        eps = wpool.tile([128, 1], F32, tag="eps", bufs=1, name="eps")
        nc.gpsimd.memset(eps[:, :], 1e-5 / (SH * SH))
        ln64 = wpool.tile([128, 1], F32, tag="ln64", bufs=1, name="ln64")
        nc.gpsimd.memset(ln64[:, :], 4.1588830833596715)

        for pr in range(bpc // 2):
            els = (2 * pr, 2 * pr + 1)
            # ---- load x and masks for both elems ----
            x_el = []
            pm2 = pools["mbc"].tile([128, PT], BF16, name="pm2")
            sm2 = pools["mbc"].tile([128, PT], BF16, name="sm2")
            pmrow2 = pools["mrow"].tile([1, PT], BF16, name="pmrow2", bufs=2)
            for el, e in enumerate(els):
                x_cs = []
                for (t0, tc) in TCH:
                    xt = pools["res"].tile([tc, E], F32, name="x_in", tag="res")
                    nc.sync.dma_start(xt[:, :], x_d[e, t0:t0 + tc, :])
                    x_cs.append(xt)
                x_el.append(x_cs)
                nc.sync.dma_start(pmrow2[0:1, el * EOFF:el * EOFF + T],
                                  pm_d[e:e + 1, :])
                nc.gpsimd.partition_broadcast(pm2[:, el * EOFF:el * EOFF + T],
                                              pmrow2[0:1, el * EOFF:el * EOFF + T])
                sm_row = pools["mrow"].tile([1, T], BF16, name="sm_row", bufs=2)
                nc.sync.dma_start(sm_row[:, :], sm_d[e:e + 1, :])
                nc.gpsimd.partition_broadcast(sm2[:, el * EOFF:el * EOFF + T],
                                              sm_row[:, :])

            # ======== self-attention ========
            h_pair = [[_layernorm(nc, pools, x_el[el][ci][:, :], tc, eps)
                       for ci, (t0, tc) in enumerate(TCH)] for el in range(2)]
            hT = _transpose_pair(nc, pools, h_pair, identb, FP8)
            qT = _project_qkT(nc, pools, wq_sa, hT, "q_sa",
                              bq_sa if has_bq_sa else None, pm2)
            kT = _project_qkT(nc, pools, wk_sa, hT, "k_sa",
                              bk_sa if has_bk_sa else None, pm2)
            v_els = [_project_v(nc, pools, wv_sa, hT, el * EOFF, "v_sa",
                                dsv_sa) for el in range(2)]
            x_el = _attention(nc, pools, (qT, kT), v_els, sel_sb, selB,
                              wo_sa, bo_sa, ones_row, x_el, exp_sa, dso_sa,
                              has_bo_sa, ln64)
            if stages == 1:
                for el, e in enumerate(els):
                    for ci, (t0, tc) in enumerate(TCH):
                        nc.sync.dma_start(out_d[e, t0:t0 + tc, :],
                                          x_el[el][ci][:, :])
                continue

            # ======== cross-attention ========
            h_pair = [[_layernorm(nc, pools, x_el[el][ci][:, :], tc, eps)
                       for ci, (t0, tc) in enumerate(TCH)] for el in range(2)]
            h2T = _transpose_pair(nc, pools, h_pair, identb, FP8)
            m_pair = []
            for el, e in enumerate(els):
                m_cs = []
                for (t0, tc) in TCH:
                    mt = pools["h"].tile([tc, E], BF16, name="m_nat",
                                         tag="m_nat", bufs=6)
                    nc.sync.dma_start(mt[:, :], mem_d[e, t0:t0 + tc, :])
                    m_cs.append(mt)
                m_pair.append(m_cs)
            mT = _transpose_pair(nc, pools, m_pair, identb, BF16)
            # masked fp8 cast: memT = (mT * SH) * sm  into DR pair views
            memT = []
            for j in range(2):
                mm = pools["tT"].tile([128, 2, PT], FP8, name="memTm", bufs=5)
                for i in range(2):
                    nc.vector.scalar_tensor_tensor(
                        mm[:, i, :], mT[j][:, i, :], SH, sm2[:, :],
                        op0=AL.mult, op1=AL.mult)
                memT.append(mm)
            qT = _project_qkT(nc, pools, wq_ca, h2T, "q_ca",
                              bq_ca if has_bq_ca else None, None)
            kT = _project_qkT(nc, pools, wk_ca, memT, "k_ca", None, None)
            v_els = [_project_v(nc, pools, wv_ca, h2T, el * EOFF, "v_ca",
                                dsv_ca) for el in range(2)]
            x_el = _attention(nc, pools, (qT, kT), v_els, sel_sb, selB,
                              wo_ca, bo_ca, ones_row, x_el, exp_ca, dso_ca,
                              has_bo_ca, ln64)
            if stages == 2:
                for el, e in enumerate(els):
                    for ci, (t0, tc) in enumerate(TCH):
                        nc.sync.dma_start(out_d[e, t0:t0 + tc, :],
                                          x_el[el][ci][:, :])
                continue

            # ======== feed-forward ========
            h_pair = [[_layernorm(nc, pools, x_el[el][ci][:, :], tc, eps)
                       for ci, (t0, tc) in enumerate(TCH)] for el in range(2)]
            h3T = _transpose_pair(nc, pools, h_pair, identb, FP8)
            rT = [pools["rT"].tile([128, 2, PT], FP8, name="r")
                  for _ in range(FCH // 2)]
            for fc in range(FCH):
                zps = pools["ps"].tile([128, PT], F32, name="z_ps",
                                       tag="ps")
                for j in range(2):
                    nc.tensor.matmul(zps[:, :],
                                     w1[:, j, :, fc * 128:(fc + 1) * 128],
                                     h3T[j][:, :, :], start=(j == 0),
                                     stop=(j == 1), perf_mode=DR)
                # relu(z)*SRT = relu(SRT*z); bias pre-scaled by SRT host-side
                nc.scalar.activation(rT[fc // 2][:, fc % 2, :], zps[:, :],
                                     AF.Relu, bias=b1c[:, fc:fc + 1],
                                     scale=relu_s)
            for el, e in enumerate(els):
                for ci, (t0, tc) in enumerate(TCH):
                    yps = pools["ps"].tile([tc, E], F32, name="y_ps",
                                           tag="ps")
                    for j in range(FCH // 2):
                        nc.tensor.matmul(
                            yps[:, :],
                            rT[j][:, :, el * EOFF + t0:el * EOFF + t0 + tc],
                            w2[:, j, :, :], start=(j == 0),
                            stop=(j == FCH // 2 - 1 and not has_b2),
                            perf_mode=DR)
                    if has_b2:
                        nc.tensor.matmul(yps[:, :], ones_row[0:1, 0:tc],
                                         b2r[0:1, :], start=False, stop=True)
                    yout = pools["res"].tile([tc, E], F32, name="yout",
                                             tag="res")
                    nc.vector.scalar_tensor_tensor(
                        yout[:, :], yps[:, :], ds2, x_el[el][ci][:, :],
                        op0=AL.mult, op1=AL.add)
                    nc.sync.dma_start(out_d[e, t0:t0 + tc, :], yout[:, :])

    nc.compile()
    return nc


def _pow2_scale(w):
    a = float(np.max(np.abs(np.asarray(w, np.float32))))
    if a == 0.0 or not np.isfinite(a):
        return 1.0
    return float(2.0 ** np.floor(np.log2(224.0 / a)))


def _weight_scales(inputs):
    g1 = np.asarray(inputs["ln1_g"], np.float32)[:, None]
    g2 = np.asarray(inputs["ln2_g"], np.float32)[:, None]
    g3 = np.asarray(inputs["ln3_g"], np.float32)[:, None]

    def wr(w):  # (H, E, D) -> [E, H*D]
        return np.transpose(np.asarray(w, np.float32), (1, 0, 2)).reshape(E, E)

    raw = {
        "wq_sa": wr(inputs["sa_wq"]), "wk_sa": wr(inputs["sa_wk"]),
        "wv_sa": wr(inputs["sa_wv"]),
        "wq_ca": wr(inputs["ca_wq"]), "wv_ca": wr(inputs["ca_wv"]),
        "w1": np.asarray(inputs["f_w1"], np.float32),
    }
    mats = {
        "wq_sa": raw["wq_sa"] * g1, "wk_sa": raw["wk_sa"] * g1,
        "wv_sa": raw["wv_sa"] * g1,
        "wo_sa": np.asarray(inputs["sa_wo"], np.float32),
        "wq_ca": raw["wq_ca"] * g2, "wk_ca": wr(inputs["ca_wk"]),
        "wv_ca": raw["wv_ca"] * g2,
        "wo_ca": np.asarray(inputs["ca_wo"], np.float32),
        "w1": raw["w1"] * g3,
        "w2": np.asarray(inputs["f_w2"], np.float32),
    }
    order = ["wq_sa", "wk_sa", "wv_sa", "wo_sa", "wq_ca", "wk_ca", "wv_ca",
             "wo_ca", "w1", "w2"]
    scales = tuple(_pow2_scale(mats[k]) for k in order)
    return mats, raw, dict(zip(order, scales)), scales


def _host_prep(inputs, mats, raw, sc, bpc, core):
    """Build the in_map for one core."""
    s = slice(core * bpc, (core + 1) * bpc)

    def dr4(m, scale):  # [E, N] -> [128, 2, 2, N] fp8 DR layout
        q = (m * scale).reshape(2, 2, 128, -1).transpose(2, 0, 1, 3)
        return np.ascontiguousarray(q).astype(NPFP8)

    def dr_w2(m, scale):  # [F, E] -> [128, 8, 2, E] fp8 DR layout
        q = (m * scale).reshape(8, 2, 128, E).transpose(2, 0, 1, 3)
        return np.ascontiguousarray(q).astype(NPFP8)

    def bcol(v, scale):  # [E] row -> [128, 4] col layout, scaled
        return np.ascontiguousarray(
            (v * scale).reshape(4, 128).T).astype(np.float32)

    def b16(a):
        return np.ascontiguousarray(np.asarray(a, np.float32)).astype(NPBF16)

    b1n = np.asarray(inputs["ln1_b"], np.float32)
    b2n = np.asarray(inputs["ln2_b"], np.float32)
    b3n = np.asarray(inputs["ln3_b"], np.float32)

    # V beta-bias folds into the output-projection bias (softmax rows sum
    # to 1, so a constant V shift passes through attention unchanged)
    bo_sa_eff = (np.asarray(inputs["sa_bo"], np.float32)
                 + (b1n @ raw["wv_sa"]) @ mats["wo_sa"])
    bo_ca_eff = (np.asarray(inputs["ca_bo"], np.float32)
                 + (b2n @ raw["wv_ca"]) @ mats["wo_ca"])

    return {
        "x": np.ascontiguousarray(np.asarray(inputs["idx"], np.float32)[s]),
        "mem": b16(inputs["memory"][s]),
        "pm": b16(np.asarray(inputs["pred_mask"])[s] != 0),
        "sm": b16(np.asarray(inputs["src_mask"])[s] != 0),
        "wq_sa": dr4(mats["wq_sa"], sc["wq_sa"]),
        "wk_sa": dr4(mats["wk_sa"], sc["wk_sa"]),
        "wv_sa": dr4(mats["wv_sa"], sc["wv_sa"]),
        "wo_sa": dr4(mats["wo_sa"], sc["wo_sa"]),
        "bo_sa": np.ascontiguousarray(
            bo_sa_eff * (SOT * sc["wo_sa"])).reshape(1, E).astype(NPBF16),
        "bq_sa": bcol(b1n @ raw["wq_sa"], SH * sc["wq_sa"]),
        "bk_sa": bcol(b1n @ raw["wk_sa"], SH * sc["wk_sa"]),
        "wq_ca": dr4(mats["wq_ca"], sc["wq_ca"]),
        "wk_ca": dr4(mats["wk_ca"], sc["wk_ca"]),
        "wv_ca": dr4(mats["wv_ca"], sc["wv_ca"]),
        "wo_ca": dr4(mats["wo_ca"], sc["wo_ca"]),
        "bo_ca": np.ascontiguousarray(
            bo_ca_eff * (SOT * sc["wo_ca"])).reshape(1, E).astype(NPBF16),
        "bq_ca": bcol(b2n @ raw["wq_ca"], SH * sc["wq_ca"]),
        "w1": dr4(mats["w1"], sc["w1"]),
        "b1": np.ascontiguousarray(
            (np.asarray(inputs["f_b1"], np.float32) + b3n @ raw["w1"])
            * SRT).reshape(1, F).astype(np.float32),
        "w2": dr_w2(mats["w2"], sc["w2"]),
        "b2": np.ascontiguousarray(
            np.asarray(inputs["f_b2"], np.float32)
            * (SRT * sc["w2"])).reshape(1, E).astype(NPBF16),
    }


def get_program(bpc, scales, flags):
    key = (bpc, scales, flags)
    if key not in _programs:
        _programs[key] = _build(bpc, scales, flags)
    return _programs[key]


def _bias_flags(inputs, mats, raw):
    b1n = np.asarray(inputs["ln1_b"], np.float32)
    b2n = np.asarray(inputs["ln2_b"], np.float32)
    nz = lambda a: bool(np.any(np.asarray(a, np.float32) != 0))
    bo_sa = (np.asarray(inputs["sa_bo"], np.float32)
             + (b1n @ raw["wv_sa"]) @ mats["wo_sa"])
    bo_ca = (np.asarray(inputs["ca_bo"], np.float32)
             + (b2n @ raw["wv_ca"]) @ mats["wo_ca"])
    return (nz(b1n @ raw["wq_sa"]), nz(b1n @ raw["wk_sa"]), nz(bo_sa),
            nz(b2n @ raw["wq_ca"]), nz(bo_ca), nz(inputs["f_b2"]))


def kernel(**inputs) -> np.ndarray:
    bpc = B // NCORES
    mats, raw, sc, scales = _weight_scales(inputs)
    flags = _bias_flags(inputs, mats, raw)
    nc = get_program(bpc, scales, flags)
    in_maps = [_host_prep(inputs, mats, raw, sc, bpc, c)
               for c in range(NCORES)]
    res = run_bass_kernel_spmd(nc, in_maps, core_ids=list(range(NCORES)))
    out = np.concatenate([res.results[c]["out"] for c in range(NCORES)], axis=0)
    return out.astype(np.float32)
